# revision 19
# baseline (speedup 1.0000x reference)
"""Trainium2 Bass kernel for nn_CrossAttentionLayer (m=n=1024, d=2048).

Math:  f = relu(term1 + term23 + term4 + ffn_b), where with
W1..W4 = ffn_w.reshape(n, 4, d) per-candidate blocks:
  term1  = sum_i u_p[i] . W1[i]                      (host: tiny scalar dot)
  term23 = <softmax_rows(S),    (W2 + u_p*W3) @ u_c.T>_F     (S = [m,n] logits)
  term4  = <softmax_rows(S.T),  (u_c*W4)      @ u_p.T>_F
Row-constant offsets cancel inside row-softmax, and the remaining column
offset folds into the matmul:  softmax_k(S[i,:]) = softmax_k((u_p[i]*w3 + w2) @ u_c.T).

Both inner products have the identical SPMD shape
  result = sum_i [ sum_k exp(T[i,k]) * M2[i,k] ] / [ sum_k exp(T[i,k]) ]
  T  = ASp @ B.T,   M2 = C @ B.T
with ASp = A*w3 + a2 and C = C1 + A*C2 folded on the host, so 8 cores run
ONE program on different operands:
  cores 0-3 (mention shard I of 256): A=u_p[I], B=u_c, C=(W2+u_p*W3)[I], a2=w2
  cores 4-7 (candidate shard J):      A=u_c[J], B=u_p, C=(u_c*W4)[J],    a2=w1
Operands are pre-transposed ([d, rows]), quantized to fp8e4m3 with static
scales (SA for ASp, SC for C; exp() descales T via the activation's scale
input, the host descales gz by SC), and packed p-major [128, chunks*cols]
so every DMA is one contiguous multi-KB run per partition.  Matmuls run in
fp8 DoubleRow perf mode (2 contraction rows per partition per cycle).
Per-core outputs are tiny gz row-vectors; the host reduces them, adds
term1 + bias, and applies relu.
"""

import sys

sys.path.insert(0, "/opt/trn_rl_repo")

import ml_dtypes
import numpy as np

import concourse.bass as bass
import concourse.tile as tile
from concourse import mybir
from concourse.bass_utils import run_bass_kernel_spmd

F32 = mybir.dt.float32
F8 = mybir.dt.float8e4
NP_F8 = ml_dtypes.float8_e4m3

M = 1024  # mentions
N = 1024  # candidates
D = 2048  # feature dim (contraction)
NCORES = 8
ISH = 256  # per-core shard rows (A rows)
CH = D // 128  # 16 contraction chunks of 128
ITILES = ISH // 128  # 2
KH = 512  # rhs free-dim per matmul (PSUM bank width in fp32)
NKH = N // KH  # 2

SA = 32.0  # ASp fp8 scale (entries ~0.03 sigma -> ~1)
SC = 512.0  # C fp8 scale (entries ~0.0015 sigma -> ~0.8)

# ---------------------------------------------------------------------------
# Workaround: the pinned neuronxcc walrus accepts fewer sync waits per
# instruction than Tile's semaphore assignment attaches.  After scheduling,
# hoist excess waits of any over-capacity instruction onto same-engine
# EventSemaphores inserted right before it; each engine executes its stream
# in order, so the waits still gate the instruction.
_DEFAULT_CAP = 1
_WAIT_CAPS = {
    "InstTensorScalarPtr": 1,
    "InstTensorScalar": 1,
    "InstScalarTensorTensor": 1,
    "InstTensorReduce": 1,
}
_wfix_counter = [0]


def _legalize_waits(nc: bass.Bass) -> None:
    for f in nc.m.functions:
        for bb in f.blocks:
            il = bb.instructions
            out = []
            for inst in il:
                si = inst.sync_info
                waits = list(si.on_wait) if si and si.on_wait else []
                cap = _WAIT_CAPS.get(type(inst).__name__, _DEFAULT_CAP)
                if len(waits) > cap:
                    keep = waits[:cap]
                    for w in waits[cap:]:
                        _wfix_counter[0] += 1
                        out.append(
                            mybir.InstEventSemaphore(
                                name=f"I-wfix-{_wfix_counter[0]}",
                                engine=inst.engine,
                                ins=[],
                                outs=[],
                                sync_info=mybir.SyncInfo(on_wait=[w], on_update=[]),
                            )
                        )
                    inst.sync_info = mybir.SyncInfo(
                        on_wait=keep, on_update=list(si.on_update or [])
                    )
                out.append(inst)
            bb.instructions = out


# ---------------------------------------------------------------------------
def _emit(nc: bass.Bass, tc: tile.TileContext, io: dict) -> None:
    mult = mybir.AluOpType.mult
    add = mybir.AluOpType.add
    DR = mybir.MatmulPerfMode.DoubleRow

    s0_r = io["s0"].ap().rearrange("p (c t k) -> p c t k", c=CH // 2, t=2)
    s1_r = io["s1"].ap().rearrange("p (c t k) -> p c t k", c=CH // 2, t=2)
    zg_r = io["out_zg"].ap().rearrange("p (c z) -> p c z", z=3)

    import contextlib

    ctx = contextlib.ExitStack()
    singles = ctx.enter_context(tc.tile_pool(name="singles", bufs=1))
    scratch = ctx.enter_context(tc.tile_pool(name="scratch", bufs=3))
    psum = ctx.enter_context(tc.tile_pool(name="psum", bufs=2, space="PSUM"))

    # stream0: per (chunk-pair, chunk): [ASp 256 | C 256 | B-kh0 512] so DMA
    # arrives in matmul consumption order with multi-KB contiguous runs.
    s0_sb = singles.tile([128, CH // 2, 2, 2 * ISH + KH], F8)
    # stream1: the B kh=1 half, same chunk order.
    s1_sb = singles.tile([128, CH // 2, 2, KH], F8)
    # zg[:, col, 0] = Z partials, zg[:, col, 1:3] = G half partials (DVE and
    # GpSimd each reduce half the columns); host divides.
    zg = singles.tile([128, ITILES * NKH, 3], F32)

    # Tensor-engine warmup: the PE array boots in a half-speed p-state and
    # reaches full clock only after ~5 us of GAPLESS execution (idle gaps
    # reset the ramp).  Burn the DMA-wait window on one junk accumulation
    # chain -- accumulating matmuls pipeline back-to-back, so the ramp
    # carries straight into the real stream that follows on the engine.
    wa_sb = singles.tile([128, 2, 128], F8)
    wb_sb = singles.tile([128, 2, 256], F8)
    nc.vector.memset(wa_sb, 0.0)
    nc.vector.memset(wb_sb, 0.0)
    NWARM = 17
    wps = psum.tile([128, 256], F32, tag="tps0")
    for w in range(NWARM):
        nc.tensor.matmul(
            wps, lhsT=wa_sb, rhs=wb_sb, start=(w == 0), stop=(w == NWARM - 1),
            perf_mode=DR,
        )

    # Input DMAs: consumption-ordered stream pieces, round-robined over the
    # three DMA-capable engines.  First pieces are small so the first
    # matmul's completion-notify (~3 us lag) lands early; later pieces are
    # big (4 KB runs per partition) for DMA throughput.
    s0_cuts = [0, 1, 2, 4, 6, 8]
    s0_eng = [nc.sync, nc.scalar, nc.gpsimd, nc.sync, nc.scalar]
    for eng, lo, hi in zip(s0_eng, s0_cuts, s0_cuts[1:]):
        eng.dma_start(out=s0_sb[:, lo:hi], in_=s0_r[:, lo:hi])
    nc.gpsimd.dma_start(out=s1_sb[:, 0:4], in_=s1_r[:, 0:4])
    nc.scalar.dma_start(out=s1_sb[:, 4:8], in_=s1_r[:, 4:8])

    # Main contraction: T and M2 accumulate over 8 DoubleRow chunks in PSUM,
    # then exp+rowsum (ACT, descaling by 1/SA) and mul+rowsum (DVE) fold k
    # away.  Each group's Z/G partial columns stream out as they finish.
    # Both it-tiles sweep each chunk together (4 matmuls per chunk pair) so
    # per-byte demand stays under the ~420 GB/s DMA supply -- a stall in the
    # tensor stream would also reset the p-state ramp.
    for kh in range(NKH):
        tps0 = psum.tile([128, KH], F32, tag="tps0")
        tps1 = psum.tile([128, KH], F32, tag="tps1")
        mps0 = psum.tile([128, KH], F32, tag="mps0")
        mps1 = psum.tile([128, KH], F32, tag="mps1")
        tps = [tps0, tps1]
        mps = [mps0, mps1]
        for c2 in range(CH // 2):
            if kh == 0:
                rhs = s0_sb[:, c2, :, 2 * ISH :]
            else:
                rhs = s1_sb[:, c2, :, :]
            for it in range(ITILES):
                isl = slice(it * 128, (it + 1) * 128)
                csl = slice(ISH + it * 128, ISH + (it + 1) * 128)
                nc.tensor.matmul(
                    tps[it],
                    lhsT=s0_sb[:, c2, :, isl],
                    rhs=rhs,
                    start=(c2 == 0),
                    stop=(c2 == CH // 2 - 1),
                    perf_mode=DR,
                )
                nc.tensor.matmul(
                    mps[it],
                    lhsT=s0_sb[:, c2, :, csl],
                    rhs=rhs,
                    start=(c2 == 0),
                    stop=(c2 == CH // 2 - 1),
                    perf_mode=DR,
                )
        for it in range(ITILES):
            col = kh * ITILES + it
            ep = scratch.tile([128, KH], F32, tag="ep")
            nc.scalar.activation(
                out=ep,
                in_=tps[it],
                func=mybir.ActivationFunctionType.Exp,
                scale=1.0 / SA,
                accum_out=zg[:, col, 0:1],
            )
            h2 = scratch.tile([128, KH], F32, tag="h")
            nc.vector.scalar_tensor_tensor(
                out=h2,
                in0=ep,
                scalar=1.0,
                in1=mps[it],
                op0=mult,
                op1=mult,
                accum_out=zg[:, col, 1:2],
            )
            nc.sync.dma_start(out=zg_r[:, col, :], in_=zg[:, col, :])
    ctx.close()


def _build() -> bass.Bass:
    nc = bass.Bass()
    io = {}
    io["s0"] = nc.declare_dram_parameter(
        "s0", [128, (CH // 2) * 2 * (2 * ISH + KH)], F8, isOutput=False
    )
    io["s1"] = nc.declare_dram_parameter(
        "s1", [128, (CH // 2) * 2 * KH], F8, isOutput=False
    )
    io["out_zg"] = nc.declare_dram_parameter(
        "out_zg", [128, 3 * ITILES * NKH], F32, isOutput=True
    )
    with tile.TileContext(nc) as tc:
        _emit(nc, tc, io)
    _legalize_waits(nc)
    return nc


_NC_CACHE: bass.Bass | None = None


def _get_nc() -> bass.Bass:
    global _NC_CACHE
    if _NC_CACHE is None:
        _NC_CACHE = _build()
    return _NC_CACHE


def _q8(a2d: np.ndarray, scale: float) -> np.ndarray:
    return np.clip(a2d * scale, -240.0, 240.0).astype(NP_F8)


def _pack_streams(asp8, c8, b8):
    """fp8 [D, 256]x2 + [D, 1024] -> (s0 [128, 8*2*1024], s1 [128, 8*2*512]):
    per partition, chunk-pair-major [ASp|C|B-kh0] blocks, then the B kh=1
    half -- the exact matmul consumption order, in contiguous runs."""
    A3 = asp8.reshape(CH // 2, 2, 128, ISH)
    C3 = c8.reshape(CH // 2, 2, 128, ISH)
    B3 = b8.reshape(CH // 2, 2, 128, N)
    s0 = np.concatenate([A3, C3, B3[..., :KH]], axis=-1)
    s0 = np.ascontiguousarray(s0.transpose(2, 0, 1, 3)).reshape(128, -1)
    s1 = np.ascontiguousarray(B3[..., KH:].transpose(2, 0, 1, 3)).reshape(128, -1)
    return s0, s1


def _in_maps(u_p, u_c, w_a, ffn_w):
    u_pT = np.ascontiguousarray(u_p.T)
    u_cT = np.ascontiguousarray(u_c.T)
    W = ffn_w.reshape(N, 4, D)
    wa = w_a[0]
    w1, w2, w3 = wa[:D], wa[D : 2 * D], wa[2 * D :]

    # host-folded operands, [d, rows]
    asp_a = u_pT * w3[:, None] + w2[:, None]
    asp_b = u_cT * w3[:, None] + w1[:, None]
    c_a = W[:, 1, :].T + u_pT * W[:, 2, :].T  # W2 + u_p*W3
    c_b = u_cT * W[:, 3, :].T  # u_c*W4

    b8_a = _q8(u_cT, 1.0)
    b8_b = _q8(u_pT, 1.0)

    maps = []
    for grp, (asp, cc, b8) in enumerate(((asp_a, c_a, b8_a), (asp_b, c_b, b8_b))):
        for ci in range(4):
            sl = slice(ISH * ci, ISH * (ci + 1))
            s0, s1 = _pack_streams(_q8(asp[:, sl], SA), _q8(cc[:, sl], SC), b8)
            maps.append({"s0": s0, "s1": s1})
    return maps


def kernel(u_p, u_c, w_a, ffn_w, ffn_b, **run_kwargs):
    nc = _get_nc()
    u_p = np.asarray(u_p, np.float32)
    u_c = np.asarray(u_c, np.float32)
    w_a = np.asarray(w_a, np.float32)
    ffn_w = np.asarray(ffn_w, np.float32)
    maps = _in_maps(u_p, u_c, w_a, ffn_w)
    res = run_bass_kernel_spmd(nc, maps, core_ids=list(range(NCORES)), **run_kwargs)
    total = 0.0
    for r in res.results:
        zg = r["out_zg"].reshape(128, NKH, ITILES, 3).astype(np.float64)
        # row (it*128+p): z = sum_kh zg[p,kh,it,0]; g = both halves + sum_kh
        z = zg[:, :, :, 0].sum(axis=1)
        g = zg[:, :, :, 1:3].sum(axis=(1, 3))
        total += (g / z).sum(dtype=np.float64)
    total /= SC
    # term1 = sum_j u_p[j] . W1[j] -- the scalar part of the final reduction
    total += float(
        np.einsum("ij,ij->", u_p, ffn_w.reshape(N, 4, D)[:, 0, :], dtype=np.float64)
    )
    f = np.float32(max(total + float(np.asarray(ffn_b)[0]), 0.0))
    out = np.array([f], dtype=np.float32)
    if run_kwargs:
        return out, res
    return out


# revision 20
# speedup vs baseline: 1.1627x; 1.1627x over previous
"""Trainium2 Bass kernel for nn_CrossAttentionLayer (m=n=1024, d=2048).

Math:  f = relu(term1 + term23 + term4 + ffn_b), where with
W1..W4 = ffn_w.reshape(n, 4, d) per-candidate blocks:
  term1  = sum_i u_p[i] . W1[i]                      (host: tiny scalar dot)
  term23 = <softmax_rows(S),    (W2 + u_p*W3) @ u_c.T>_F     (S = [m,n] logits)
  term4  = <softmax_rows(S.T),  (u_c*W4)      @ u_p.T>_F
Row-constant offsets cancel inside row-softmax, and the remaining column
offset folds into the matmul:  softmax_k(S[i,:]) = softmax_k((u_p[i]*w3 + w2) @ u_c.T).

Both inner products have the identical SPMD shape
  result = sum_i [ sum_k exp(T[i,k]) * M2[i,k] ] / [ sum_k exp(T[i,k]) ]
  T  = ASp @ B.T,   M2 = C @ B.T
with ASp = A*w3 + a2 and C = C1 + A*C2 folded on the host, so 8 cores run
ONE program on different operands:
  cores 0-3 (mention shard I of 256): A=u_p[I], B=u_c, C=(W2+u_p*W3)[I], a2=w2
  cores 4-7 (candidate shard J):      A=u_c[J], B=u_p, C=(u_c*W4)[J],    a2=w1
Operands are pre-transposed ([d, rows]), quantized to fp8e4m3 with static
scales (SA for ASp, SC for C; exp() descales T via the activation's scale
input, the host descales gz by SC), and packed p-major [128, chunks*cols]
so every DMA is one contiguous multi-KB run per partition.  Matmuls run in
fp8 DoubleRow perf mode (2 contraction rows per partition per cycle).
Per-core outputs are tiny gz row-vectors; the host reduces them, adds
term1 + bias, and applies relu.
"""

import sys

sys.path.insert(0, "/opt/trn_rl_repo")

import ml_dtypes
import numpy as np

import concourse.bass as bass
import concourse.tile as tile
from concourse import mybir
from concourse.bass_utils import run_bass_kernel_spmd

F32 = mybir.dt.float32
F8 = mybir.dt.float8e4
NP_F8 = ml_dtypes.float8_e4m3

M = 1024  # mentions
N = 1024  # candidates
D = 2048  # feature dim (contraction)
NCORES = 8
ISH = 256  # per-core shard rows (A rows)
CH = D // 128  # 16 contraction chunks of 128
ITILES = ISH // 128  # 2
KH = 512  # rhs free-dim per matmul (PSUM bank width in fp32)
NKH = N // KH  # 2

SA = 32.0  # ASp fp8 scale (entries ~0.03 sigma -> ~1)
SC = 512.0  # C fp8 scale (entries ~0.0015 sigma -> ~0.8)

# ---------------------------------------------------------------------------
# Workaround: the pinned neuronxcc walrus accepts fewer sync waits per
# instruction than Tile's semaphore assignment attaches.  After scheduling,
# hoist excess waits of any over-capacity instruction onto same-engine
# EventSemaphores inserted right before it; each engine executes its stream
# in order, so the waits still gate the instruction.
_DEFAULT_CAP = 1
_WAIT_CAPS = {
    "InstTensorScalarPtr": 1,
    "InstTensorScalar": 1,
    "InstScalarTensorTensor": 1,
    "InstTensorReduce": 1,
}
_wfix_counter = [0]


def _legalize_waits(nc: bass.Bass) -> None:
    for f in nc.m.functions:
        for bb in f.blocks:
            il = bb.instructions
            out = []
            for inst in il:
                si = inst.sync_info
                waits = list(si.on_wait) if si and si.on_wait else []
                cap = _WAIT_CAPS.get(type(inst).__name__, _DEFAULT_CAP)
                if len(waits) > cap:
                    keep = waits[:cap]
                    for w in waits[cap:]:
                        _wfix_counter[0] += 1
                        out.append(
                            mybir.InstEventSemaphore(
                                name=f"I-wfix-{_wfix_counter[0]}",
                                engine=inst.engine,
                                ins=[],
                                outs=[],
                                sync_info=mybir.SyncInfo(on_wait=[w], on_update=[]),
                            )
                        )
                    inst.sync_info = mybir.SyncInfo(
                        on_wait=keep, on_update=list(si.on_update or [])
                    )
                out.append(inst)
            bb.instructions = out


# ---------------------------------------------------------------------------
def _emit(nc: bass.Bass, tc: tile.TileContext, io: dict) -> None:
    mult = mybir.AluOpType.mult
    add = mybir.AluOpType.add
    DR = mybir.MatmulPerfMode.DoubleRow

    s0_r = io["s0"].ap().rearrange("p (c t k) -> p c t k", c=CH // 2, t=2)
    s1_r = io["s1"].ap().rearrange("p (c t k) -> p c t k", c=CH // 2, t=2)
    zg_r = io["out_zg"].ap().rearrange("p (c z) -> p c z", z=3)

    import contextlib

    ctx = contextlib.ExitStack()
    singles = ctx.enter_context(tc.tile_pool(name="singles", bufs=1))
    scratch = ctx.enter_context(tc.tile_pool(name="scratch", bufs=3))
    psum = ctx.enter_context(tc.tile_pool(name="psum", bufs=2, space="PSUM"))

    # stream0: per (chunk-pair, chunk): [ASp 256 | C 256 | B-kh0 512] so DMA
    # arrives in matmul consumption order with multi-KB contiguous runs.
    s0_sb = singles.tile([128, CH // 2, 2, 2 * ISH + KH], F8)
    # stream1: the B kh=1 half, same chunk order.
    s1_sb = singles.tile([128, CH // 2, 2, KH], F8)
    # zg[:, col, 0] = Z partials, zg[:, col, 1:3] = G half partials (DVE and
    # GpSimd each reduce half the columns); host divides.
    zg = singles.tile([128, ITILES * NKH, 3], F32)

    # Tensor-engine warmup: the PE array boots in a half-speed p-state and
    # reaches full clock only after ~5 us of GAPLESS execution (idle gaps
    # reset the ramp).  Burn the DMA-wait window on one junk accumulation
    # chain -- accumulating matmuls pipeline back-to-back, so the ramp
    # carries straight into the real stream that follows on the engine.
    wa_sb = singles.tile([128, 2, 128], F8)
    wb_sb = singles.tile([128, 2, 256], F8)
    nc.vector.memset(wa_sb, 0.0)
    nc.vector.memset(wb_sb, 0.0)
    NWARM = 17
    wps = psum.tile([128, 256], F32, tag="tps0")
    for w in range(NWARM):
        nc.tensor.matmul(
            wps, lhsT=wa_sb, rhs=wb_sb, start=(w == 0), stop=(w == NWARM - 1),
            perf_mode=DR,
        )

    # Input DMAs: consumption-ordered stream pieces, round-robined over the
    # three DMA-capable engines.  First pieces are small so the first
    # matmul's completion-notify (~3 us lag) lands early; later pieces are
    # big (4 KB runs per partition) for DMA throughput.
    engs = [nc.sync, nc.scalar, nc.gpsimd]
    s0_cuts = [0, 1, 2, 3, 4, 5, 6, 8]
    for j, (lo, hi) in enumerate(zip(s0_cuts, s0_cuts[1:])):
        engs[j % 3].dma_start(out=s0_sb[:, lo:hi], in_=s0_r[:, lo:hi])
    s1_cuts = [0, 2, 4, 6, 8]
    for j, (lo, hi) in enumerate(zip(s1_cuts, s1_cuts[1:])):
        engs[(j + 1) % 3].dma_start(out=s1_sb[:, lo:hi], in_=s1_r[:, lo:hi])

    # Main contraction: T and M2 accumulate over 8 DoubleRow chunks in PSUM,
    # then exp+rowsum (ACT, descaling by 1/SA) and mul+rowsum (DVE) fold k
    # away.  Each group's Z/G partial columns stream out as they finish.
    # Both it-tiles sweep each chunk together (4 matmuls per chunk pair) so
    # per-byte demand stays under the ~420 GB/s DMA supply -- a stall in the
    # tensor stream would also reset the p-state ramp.
    for kh in range(NKH):
        tps0 = psum.tile([128, KH], F32, tag="tps0")
        tps1 = psum.tile([128, KH], F32, tag="tps1")
        mps0 = psum.tile([128, KH], F32, tag="mps0")
        mps1 = psum.tile([128, KH], F32, tag="mps1")
        tps = [tps0, tps1]
        mps = [mps0, mps1]
        for c2 in range(CH // 2):
            if kh == 0:
                rhs = s0_sb[:, c2, :, 2 * ISH :]
            else:
                rhs = s1_sb[:, c2, :, :]
            for it in range(ITILES):
                isl = slice(it * 128, (it + 1) * 128)
                csl = slice(ISH + it * 128, ISH + (it + 1) * 128)
                nc.tensor.matmul(
                    tps[it],
                    lhsT=s0_sb[:, c2, :, isl],
                    rhs=rhs,
                    start=(c2 == 0),
                    stop=(c2 == CH // 2 - 1),
                    perf_mode=DR,
                )
                nc.tensor.matmul(
                    mps[it],
                    lhsT=s0_sb[:, c2, :, csl],
                    rhs=rhs,
                    start=(c2 == 0),
                    stop=(c2 == CH // 2 - 1),
                    perf_mode=DR,
                )
        for it in range(ITILES):
            col = kh * ITILES + it
            ep = scratch.tile([128, KH], F32, tag="ep")
            nc.scalar.activation(
                out=ep,
                in_=tps[it],
                func=mybir.ActivationFunctionType.Exp,
                scale=1.0 / SA,
                accum_out=zg[:, col, 0:1],
            )
            h2 = scratch.tile([128, KH], F32, tag="h")
            nc.vector.scalar_tensor_tensor(
                out=h2,
                in0=ep,
                scalar=1.0,
                in1=mps[it],
                op0=mult,
                op1=mult,
                accum_out=zg[:, col, 1:2],
            )
            nc.sync.dma_start(out=zg_r[:, col, :], in_=zg[:, col, :])
    ctx.close()


def _build() -> bass.Bass:
    nc = bass.Bass()
    io = {}
    io["s0"] = nc.declare_dram_parameter(
        "s0", [128, (CH // 2) * 2 * (2 * ISH + KH)], F8, isOutput=False
    )
    io["s1"] = nc.declare_dram_parameter(
        "s1", [128, (CH // 2) * 2 * KH], F8, isOutput=False
    )
    io["out_zg"] = nc.declare_dram_parameter(
        "out_zg", [128, 3 * ITILES * NKH], F32, isOutput=True
    )
    with tile.TileContext(nc) as tc:
        _emit(nc, tc, io)
    _legalize_waits(nc)
    return nc


_NC_CACHE: bass.Bass | None = None


def _get_nc() -> bass.Bass:
    global _NC_CACHE
    if _NC_CACHE is None:
        _NC_CACHE = _build()
    return _NC_CACHE


def _q8(a2d: np.ndarray, scale: float) -> np.ndarray:
    return np.clip(a2d * scale, -240.0, 240.0).astype(NP_F8)


def _pack_streams(asp8, c8, b8):
    """fp8 [D, 256]x2 + [D, 1024] -> (s0 [128, 8*2*1024], s1 [128, 8*2*512]):
    per partition, chunk-pair-major [ASp|C|B-kh0] blocks, then the B kh=1
    half -- the exact matmul consumption order, in contiguous runs."""
    A3 = asp8.reshape(CH // 2, 2, 128, ISH)
    C3 = c8.reshape(CH // 2, 2, 128, ISH)
    B3 = b8.reshape(CH // 2, 2, 128, N)
    s0 = np.concatenate([A3, C3, B3[..., :KH]], axis=-1)
    s0 = np.ascontiguousarray(s0.transpose(2, 0, 1, 3)).reshape(128, -1)
    s1 = np.ascontiguousarray(B3[..., KH:].transpose(2, 0, 1, 3)).reshape(128, -1)
    return s0, s1


def _in_maps(u_p, u_c, w_a, ffn_w):
    u_pT = np.ascontiguousarray(u_p.T)
    u_cT = np.ascontiguousarray(u_c.T)
    W = ffn_w.reshape(N, 4, D)
    wa = w_a[0]
    w1, w2, w3 = wa[:D], wa[D : 2 * D], wa[2 * D :]

    # host-folded operands, [d, rows]
    asp_a = u_pT * w3[:, None] + w2[:, None]
    asp_b = u_cT * w3[:, None] + w1[:, None]
    c_a = W[:, 1, :].T + u_pT * W[:, 2, :].T  # W2 + u_p*W3
    c_b = u_cT * W[:, 3, :].T  # u_c*W4

    b8_a = _q8(u_cT, 1.0)
    b8_b = _q8(u_pT, 1.0)

    maps = []
    for grp, (asp, cc, b8) in enumerate(((asp_a, c_a, b8_a), (asp_b, c_b, b8_b))):
        for ci in range(4):
            sl = slice(ISH * ci, ISH * (ci + 1))
            s0, s1 = _pack_streams(_q8(asp[:, sl], SA), _q8(cc[:, sl], SC), b8)
            maps.append({"s0": s0, "s1": s1})
    return maps


def kernel(u_p, u_c, w_a, ffn_w, ffn_b, **run_kwargs):
    nc = _get_nc()
    u_p = np.asarray(u_p, np.float32)
    u_c = np.asarray(u_c, np.float32)
    w_a = np.asarray(w_a, np.float32)
    ffn_w = np.asarray(ffn_w, np.float32)
    maps = _in_maps(u_p, u_c, w_a, ffn_w)
    res = run_bass_kernel_spmd(nc, maps, core_ids=list(range(NCORES)), **run_kwargs)
    total = 0.0
    for r in res.results:
        zg = r["out_zg"].reshape(128, NKH, ITILES, 3).astype(np.float64)
        # row (it*128+p): z = sum_kh zg[p,kh,it,0]; g = both halves + sum_kh
        z = zg[:, :, :, 0].sum(axis=1)
        g = zg[:, :, :, 1:3].sum(axis=(1, 3))
        total += (g / z).sum(dtype=np.float64)
    total /= SC
    # term1 = sum_j u_p[j] . W1[j] -- the scalar part of the final reduction
    total += float(
        np.einsum("ij,ij->", u_p, ffn_w.reshape(N, 4, D)[:, 0, :], dtype=np.float64)
    )
    f = np.float32(max(total + float(np.asarray(ffn_b)[0]), 0.0))
    out = np.array([f], dtype=np.float32)
    if run_kwargs:
        return out, res
    return out


# revision 21
# speedup vs baseline: 1.2206x; 1.0498x over previous
"""Trainium2 Bass kernel for nn_CrossAttentionLayer (m=n=1024, d=2048).

Math:  f = relu(term1 + term23 + term4 + ffn_b), where with
W1..W4 = ffn_w.reshape(n, 4, d) per-candidate blocks:
  term1  = sum_i u_p[i] . W1[i]                      (host: tiny scalar dot)
  term23 = <softmax_rows(S),    (W2 + u_p*W3) @ u_c.T>_F     (S = [m,n] logits)
  term4  = <softmax_rows(S.T),  (u_c*W4)      @ u_p.T>_F
Row-constant offsets cancel inside row-softmax, and the remaining column
offset folds into the matmul:  softmax_k(S[i,:]) = softmax_k((u_p[i]*w3 + w2) @ u_c.T).

Both inner products have the identical SPMD shape
  result = sum_i [ sum_k exp(T[i,k]) * M2[i,k] ] / [ sum_k exp(T[i,k]) ]
  T  = ASp @ B.T,   M2 = C @ B.T
with ASp = A*w3 + a2 and C = C1 + A*C2 folded on the host, so 8 cores run
ONE program on different operands:
  cores 0-3 (mention shard I of 256): A=u_p[I], B=u_c, C=(W2+u_p*W3)[I], a2=w2
  cores 4-7 (candidate shard J):      A=u_c[J], B=u_p, C=(u_c*W4)[J],    a2=w1
Operands are pre-transposed ([d, rows]), quantized to fp8e4m3 with static
scales (SA for ASp, SC for C; exp() descales T via the activation's scale
input, the host descales gz by SC), and packed p-major [128, chunks*cols]
so every DMA is one contiguous multi-KB run per partition.  Matmuls run in
fp8 DoubleRow perf mode (2 contraction rows per partition per cycle).
Per-core outputs are tiny gz row-vectors; the host reduces them, adds
term1 + bias, and applies relu.
"""

import sys

sys.path.insert(0, "/opt/trn_rl_repo")

import ml_dtypes
import numpy as np

import concourse.bass as bass
import concourse.tile as tile
from concourse import mybir
from concourse.bass_utils import run_bass_kernel_spmd

F32 = mybir.dt.float32
F8 = mybir.dt.float8e4
NP_F8 = ml_dtypes.float8_e4m3

M = 1024  # mentions
N = 1024  # candidates
D = 2048  # feature dim (contraction)
NCORES = 8
ISH = 256  # per-core shard rows (A rows)
CH = D // 128  # 16 contraction chunks of 128
ITILES = ISH // 128  # 2
KH = 512  # rhs free-dim per matmul (PSUM bank width in fp32)
NKH = N // KH  # 2

SA = 32.0  # ASp fp8 scale (entries ~0.03 sigma -> ~1)
SC = 512.0  # C fp8 scale (entries ~0.0015 sigma -> ~0.8)

# ---------------------------------------------------------------------------
# Workaround: the pinned neuronxcc walrus accepts fewer sync waits per
# instruction than Tile's semaphore assignment attaches.  After scheduling,
# hoist excess waits of any over-capacity instruction onto same-engine
# EventSemaphores inserted right before it; each engine executes its stream
# in order, so the waits still gate the instruction.
_DEFAULT_CAP = 1
_WAIT_CAPS = {
    "InstTensorScalarPtr": 1,
    "InstTensorScalar": 1,
    "InstScalarTensorTensor": 1,
    "InstTensorReduce": 1,
}
_wfix_counter = [0]


def _legalize_waits(nc: bass.Bass) -> None:
    for f in nc.m.functions:
        for bb in f.blocks:
            il = bb.instructions
            out = []
            for inst in il:
                si = inst.sync_info
                waits = list(si.on_wait) if si and si.on_wait else []
                cap = _WAIT_CAPS.get(type(inst).__name__, _DEFAULT_CAP)
                if len(waits) > cap:
                    keep = waits[:cap]
                    for w in waits[cap:]:
                        _wfix_counter[0] += 1
                        out.append(
                            mybir.InstEventSemaphore(
                                name=f"I-wfix-{_wfix_counter[0]}",
                                engine=inst.engine,
                                ins=[],
                                outs=[],
                                sync_info=mybir.SyncInfo(on_wait=[w], on_update=[]),
                            )
                        )
                    inst.sync_info = mybir.SyncInfo(
                        on_wait=keep, on_update=list(si.on_update or [])
                    )
                out.append(inst)
            bb.instructions = out


# ---------------------------------------------------------------------------
def _emit(nc: bass.Bass, tc: tile.TileContext, io: dict) -> None:
    mult = mybir.AluOpType.mult
    add = mybir.AluOpType.add
    DR = mybir.MatmulPerfMode.DoubleRow

    s0_r = io["s0"].ap().rearrange("p (c t k) -> p c t k", c=CH // 2, t=2)
    s1_r = io["s1"].ap().rearrange("p (c t k) -> p c t k", c=CH // 2, t=2)
    zg_r = io["out_zg"].ap().rearrange("p (c z) -> p c z", z=3)

    import contextlib

    ctx = contextlib.ExitStack()
    singles = ctx.enter_context(tc.tile_pool(name="singles", bufs=1))
    scratch = ctx.enter_context(tc.tile_pool(name="scratch", bufs=3))
    psum = ctx.enter_context(tc.tile_pool(name="psum", bufs=2, space="PSUM"))

    # stream0: per (chunk-pair, chunk): [ASp 256 | C 256 | B-kh0 512] so DMA
    # arrives in matmul consumption order with multi-KB contiguous runs.
    s0_sb = singles.tile([128, CH // 2, 2, 2 * ISH + KH], F8)
    # stream1: the B kh=1 half, same chunk order.
    s1_sb = singles.tile([128, CH // 2, 2, KH], F8)
    # zg[:, col, 0] = Z partials, zg[:, col, 1:3] = G half partials (DVE and
    # GpSimd each reduce half the columns); host divides.
    zg = singles.tile([128, ITILES * NKH, 3], F32)

    # Tensor-engine warmup: the PE array boots in a half-speed p-state and
    # reaches full clock only after ~5 us of GAPLESS execution (idle gaps
    # reset the ramp).  Burn the DMA-wait window on one junk accumulation
    # chain -- accumulating matmuls pipeline back-to-back, so the ramp
    # carries straight into the real stream that follows on the engine.
    wa_sb = singles.tile([128, 2, 128], F8)
    wb_sb = singles.tile([128, 2, 256], F8)
    nc.vector.memset(wa_sb, 0.0)
    nc.vector.memset(wb_sb, 0.0)
    NWARM = 17
    wps = psum.tile([128, 256], F32, tag="tps0")
    for w in range(NWARM):
        nc.tensor.matmul(
            wps, lhsT=wa_sb, rhs=wb_sb, start=(w == 0), stop=(w == NWARM - 1),
            perf_mode=DR,
        )

    # Input DMAs: consumption-ordered stream pieces, round-robined over the
    # three DMA-capable engines.  First pieces are small so the first
    # matmul's completion-notify (~3 us lag) lands early; later pieces are
    # big (4 KB runs per partition) for DMA throughput.
    # Round-robin s0's pieces so all queues drain their s0 share at the same
    # time, then append s1 -- keeps the late-needed kh=1 bytes from stealing
    # bandwidth while the kh=0 sweep is still consuming.
    engs = [nc.sync, nc.scalar, nc.gpsimd]
    for j in range(CH // 2):
        engs[j % 3].dma_start(out=s0_sb[:, j : j + 1], in_=s0_r[:, j : j + 1])
    s1_asn = [(nc.gpsimd, 0, 2), (nc.gpsimd, 2, 4), (nc.sync, 4, 6), (nc.scalar, 6, 8)]
    for eng, lo, hi in s1_asn:
        eng.dma_start(out=s1_sb[:, lo:hi], in_=s1_r[:, lo:hi])

    # Main contraction: T and M2 accumulate over 8 DoubleRow chunks in PSUM,
    # then exp+rowsum (ACT, descaling by 1/SA) and mul+rowsum (DVE) fold k
    # away.  Each group's Z/G partial columns stream out as they finish.
    # Both it-tiles sweep each chunk together (4 matmuls per chunk pair) so
    # per-byte demand stays under the ~420 GB/s DMA supply -- a stall in the
    # tensor stream would also reset the p-state ramp.
    for kh in range(NKH):
        tps0 = psum.tile([128, KH], F32, tag="tps0")
        tps1 = psum.tile([128, KH], F32, tag="tps1")
        mps0 = psum.tile([128, KH], F32, tag="mps0")
        mps1 = psum.tile([128, KH], F32, tag="mps1")
        tps = [tps0, tps1]
        mps = [mps0, mps1]
        for c2 in range(CH // 2):
            if kh == 0:
                rhs = s0_sb[:, c2, :, 2 * ISH :]
            else:
                rhs = s1_sb[:, c2, :, :]
            for it in range(ITILES):
                isl = slice(it * 128, (it + 1) * 128)
                csl = slice(ISH + it * 128, ISH + (it + 1) * 128)
                nc.tensor.matmul(
                    tps[it],
                    lhsT=s0_sb[:, c2, :, isl],
                    rhs=rhs,
                    start=(c2 == 0),
                    stop=(c2 == CH // 2 - 1),
                    perf_mode=DR,
                )
                nc.tensor.matmul(
                    mps[it],
                    lhsT=s0_sb[:, c2, :, csl],
                    rhs=rhs,
                    start=(c2 == 0),
                    stop=(c2 == CH // 2 - 1),
                    perf_mode=DR,
                )
        for it in range(ITILES):
            col = kh * ITILES + it
            ep = scratch.tile([128, KH], F32, tag="ep")
            nc.scalar.activation(
                out=ep,
                in_=tps[it],
                func=mybir.ActivationFunctionType.Exp,
                scale=1.0 / SA,
                accum_out=zg[:, col, 0:1],
            )
            h2 = scratch.tile([128, KH], F32, tag="h")
            nc.vector.scalar_tensor_tensor(
                out=h2,
                in0=ep,
                scalar=1.0,
                in1=mps[it],
                op0=mult,
                op1=mult,
                accum_out=zg[:, col, 1:2],
            )
            nc.sync.dma_start(out=zg_r[:, col, :], in_=zg[:, col, :])
    ctx.close()


def _build() -> bass.Bass:
    nc = bass.Bass()
    io = {}
    io["s0"] = nc.declare_dram_parameter(
        "s0", [128, (CH // 2) * 2 * (2 * ISH + KH)], F8, isOutput=False
    )
    io["s1"] = nc.declare_dram_parameter(
        "s1", [128, (CH // 2) * 2 * KH], F8, isOutput=False
    )
    io["out_zg"] = nc.declare_dram_parameter(
        "out_zg", [128, 3 * ITILES * NKH], F32, isOutput=True
    )
    with tile.TileContext(nc) as tc:
        _emit(nc, tc, io)
    _legalize_waits(nc)
    return nc


_NC_CACHE: bass.Bass | None = None


def _get_nc() -> bass.Bass:
    global _NC_CACHE
    if _NC_CACHE is None:
        _NC_CACHE = _build()
    return _NC_CACHE


def _q8(a2d: np.ndarray, scale: float) -> np.ndarray:
    return np.clip(a2d * scale, -240.0, 240.0).astype(NP_F8)


def _pack_streams(asp8, c8, b8):
    """fp8 [D, 256]x2 + [D, 1024] -> (s0 [128, 8*2*1024], s1 [128, 8*2*512]):
    per partition, chunk-pair-major [ASp|C|B-kh0] blocks, then the B kh=1
    half -- the exact matmul consumption order, in contiguous runs."""
    A3 = asp8.reshape(CH // 2, 2, 128, ISH)
    C3 = c8.reshape(CH // 2, 2, 128, ISH)
    B3 = b8.reshape(CH // 2, 2, 128, N)
    s0 = np.concatenate([A3, C3, B3[..., :KH]], axis=-1)
    s0 = np.ascontiguousarray(s0.transpose(2, 0, 1, 3)).reshape(128, -1)
    s1 = np.ascontiguousarray(B3[..., KH:].transpose(2, 0, 1, 3)).reshape(128, -1)
    return s0, s1


def _in_maps(u_p, u_c, w_a, ffn_w):
    u_pT = np.ascontiguousarray(u_p.T)
    u_cT = np.ascontiguousarray(u_c.T)
    W = ffn_w.reshape(N, 4, D)
    wa = w_a[0]
    w1, w2, w3 = wa[:D], wa[D : 2 * D], wa[2 * D :]

    # host-folded operands, [d, rows]
    asp_a = u_pT * w3[:, None] + w2[:, None]
    asp_b = u_cT * w3[:, None] + w1[:, None]
    c_a = W[:, 1, :].T + u_pT * W[:, 2, :].T  # W2 + u_p*W3
    c_b = u_cT * W[:, 3, :].T  # u_c*W4

    b8_a = _q8(u_cT, 1.0)
    b8_b = _q8(u_pT, 1.0)

    maps = []
    for grp, (asp, cc, b8) in enumerate(((asp_a, c_a, b8_a), (asp_b, c_b, b8_b))):
        for ci in range(4):
            sl = slice(ISH * ci, ISH * (ci + 1))
            s0, s1 = _pack_streams(_q8(asp[:, sl], SA), _q8(cc[:, sl], SC), b8)
            maps.append({"s0": s0, "s1": s1})
    return maps


def kernel(u_p, u_c, w_a, ffn_w, ffn_b, **run_kwargs):
    nc = _get_nc()
    u_p = np.asarray(u_p, np.float32)
    u_c = np.asarray(u_c, np.float32)
    w_a = np.asarray(w_a, np.float32)
    ffn_w = np.asarray(ffn_w, np.float32)
    maps = _in_maps(u_p, u_c, w_a, ffn_w)
    res = run_bass_kernel_spmd(nc, maps, core_ids=list(range(NCORES)), **run_kwargs)
    total = 0.0
    for r in res.results:
        zg = r["out_zg"].reshape(128, NKH, ITILES, 3).astype(np.float64)
        # row (it*128+p): z = sum_kh zg[p,kh,it,0]; g = both halves + sum_kh
        z = zg[:, :, :, 0].sum(axis=1)
        g = zg[:, :, :, 1:3].sum(axis=(1, 3))
        total += (g / z).sum(dtype=np.float64)
    total /= SC
    # term1 = sum_j u_p[j] . W1[j] -- the scalar part of the final reduction
    total += float(
        np.einsum("ij,ij->", u_p, ffn_w.reshape(N, 4, D)[:, 0, :], dtype=np.float64)
    )
    f = np.float32(max(total + float(np.asarray(ffn_b)[0]), 0.0))
    out = np.array([f], dtype=np.float32)
    if run_kwargs:
        return out, res
    return out
